# revision 10
# baseline (speedup 1.0000x reference)
# Trainium2 Bass kernel for nn_Critic (3-node GAT x2 + LayerNorm MLP critic).
#
# Strategy: pure data parallel over 8 NeuronCores (batch 262144 -> 32768/core).
# Per core, samples are processed 512 at a time (4 subtiles of 128). All
# matmuls run on the TensorEngine with activations-as-lhsT (batch on PSUM
# partitions), attention softmax/aggregation runs batch-major on DVE with
# per-sample scalars on partitions, LayerNorm+LeakyReLU fuses into a single
# ACT op via scale/bias, and all weight preprocessing happens host-side:
#   - GAT z-projection + attention scores come from ONE matmul per subtile
#     against a [W block-diag | W@a_src | W@a_dst] table (scores s = z.a =
#     x.(W a) need no z evacuation).
#   - biases ride as an extra ones-row in lhsT / bias-row in the table.
#   - elu(x) is computed as min(exp x, 1) + relu(x) = elu(x)+1; the constant
#     +1 is cancelled exactly by subtracting colsum(W2) from the next
#     layer's bias row.
#   - rsqrt(v+eps) = exp(-0.5*ln(v+eps)) so every ACT function used lives in
#     the single 'natural_log_exp_and_others' table set (no table switches).

import numpy as np

B, N, OBS, ACTD, H = 262144, 3, 30, 5, 64
IN = OBS + ACTD  # 35
NCORES = 8
NEG_ATT = 0.2
NEG_MLP = 0.01
LN_EPS = 1e-5

_prog_cache = {}


def _split_excess_waits(nc, max_waits=1):
    """This walrus build rejects >1 sync wait per instruction; hoist excess
    waits onto preceding same-engine NOPs."""
    import bass_rust
    import concourse.mybir as mybir

    n_split = 0
    for f in nc.m.functions:
        for bb in f.blocks:
            insts = bb.instructions
            out = []
            changed = False
            for ins in insts:
                si = ins.sync_info
                waits = list(si.on_wait) if si is not None and si.on_wait else []
                if len(waits) > max_waits:
                    changed = True
                    extra, keep = waits[:-max_waits], waits[-max_waits:]
                    for ci in range(0, len(extra), max_waits):
                        chunk = extra[ci : ci + max_waits]
                        nop = mybir.InstNoOp(
                            name=f"{ins.name}-wsplit{ci}", engine=ins.engine
                        )
                        nop.sync_info = bass_rust.SyncInfo(on_wait=chunk, on_update=[])
                        out.append(nop)
                        n_split += 1
                    ups = list(si.on_update) if si.on_update else []
                    ins.sync_info = bass_rust.SyncInfo(on_wait=keep, on_update=ups)
                out.append(ins)
            if changed:
                bb.instructions = out
    return n_split


def _build_program(Bc, split_waits=True):
    """Build the per-core Bass program. Bc must be a multiple of 512."""
    from contextlib import ExitStack

    import concourse.bass as bass
    import concourse.mybir as mybir
    import concourse.tile as tile

    f32 = mybir.dt.float32
    i32 = mybir.dt.int32
    AF = mybir.ActivationFunctionType
    OP = mybir.AluOpType
    AX = mybir.AxisListType

    T = Bc // 512  # tiles of 512 samples
    OUTC = 4 * T  # columns in the output accumulator

    nc = bass.Bass("TRN2", target_bir_lowering=False, debug=False)

    obs = nc.dram_tensor("obs", [Bc, N * OBS], f32, kind="ExternalInput")
    act = nc.dram_tensor("act", [Bc, N * ACTD], f32, kind="ExternalInput")
    adj = nc.dram_tensor("adj", [Bc, N * N], i32, kind="ExternalInput")
    w1a = nc.dram_tensor("w1a", [IN * N + 1, 198], f32, kind="ExternalInput")
    w2aa = nc.dram_tensor("w2aa", [128, 198], f32, kind="ExternalInput")
    w2ab = nc.dram_tensor("w2ab", [65, 198], f32, kind="ExternalInput")
    wo1a = nc.dram_tensor("wo1a", [128, 256], f32, kind="ExternalInput")
    wo1b = nc.dram_tensor("wo1b", [64, 256], f32, kind="ExternalInput")
    wo2a = nc.dram_tensor("wo2a", [128, 128], f32, kind="ExternalInput")
    wo2b = nc.dram_tensor("wo2b", [128, 128], f32, kind="ExternalInput")
    wo3r = nc.dram_tensor("wo3r", [128, 128], f32, kind="ExternalInput")
    ident = nc.dram_tensor("ident", [128, 128], f32, kind="ExternalInput")
    out = nc.dram_tensor("out", [Bc, 1], f32, kind="ExternalOutput")

    with tile.TileContext(nc) as tc:
        with ExitStack() as ctx:
            P = lambda **kw: ctx.enter_context(tc.tile_pool(**kw))
            cpool = P(name="consts", bufs=1)
            xpool = P(name="x", bufs=2)
            ipool = P(name="adj", bufs=2)
            xtpool = P(name="xt", bufs=3)
            apool = P(name="attn", bufs=2)
            hpool = P(name="h", bufs=2)
            epool = P(name="elu", bufs=2)
            htpool = P(name="ht", bufs=2)
            ypool = P(name="y", bufs=2)
            spool = P(name="stats", bufs=3)
            opool = P(name="outacc", bufs=1)
            # PSUM: 3 + 3 + 2 banks = 8
            ps_t = P(name="ps_t", bufs=3, space="PSUM")
            ps_z = P(name="ps_z", bufs=3, space="PSUM")
            ps_u = P(name="ps_u", bufs=2, space="PSUM")

            # ---- constants ----
            def cload(dt_, dram, shape, dtype=f32):
                t = cpool.tile(shape, dtype, tag=dt_)
                nc.gpsimd.dma_start(t[:], dram[:, :])
                return t

            c_id = cload("c_id", ident, [128, 128])
            c_w1a = cload("c_w1a", w1a, [IN * N + 1, 198])
            c_w2aa = cload("c_w2aa", w2aa, [128, 198])
            c_w2ab = cload("c_w2ab", w2ab, [65, 198])
            c_wo1a = cload("c_wo1a", wo1a, [128, 256])
            c_wo1b = cload("c_wo1b", wo1b, [64, 256])
            c_wo2a = cload("c_wo2a", wo2a, [128, 128])
            c_wo2b = cload("c_wo2b", wo2b, [128, 128])
            c_wo3r = cload("c_wo3r", wo3r, [128, 128])
            c_eps = cpool.tile([128, 1], f32, tag="c_eps")
            nc.gpsimd.memset(c_eps[:, :], LN_EPS)

            outacc = opool.tile([128, OUTC], f32)

            obs_r = obs.rearrange("(t s p) j -> t p s j", s=4, p=128)
            act_r = act.rearrange("(t s p) j -> t p s j", s=4, p=128)
            adj_r = adj.rearrange("(t s p) j -> t p s j", s=4, p=128)

            for t in range(T):
                # ---- load x batch-major: per sub 107 cols = obs 90|act 15|pad|one
                xbm = xpool.tile([128, 4 * 106], f32, tag="xbm")
                xbm_v = xbm[:, :].rearrange("p (s c) -> p s c", s=4)
                nc.gpsimd.dma_start(xbm_v[:, :, 0:90], obs_r[t])
                nc.gpsimd.dma_start(xbm_v[:, :, 90:105], act_r[t])
                nc.gpsimd.memset(xbm_v[:, :, 105:106], 1.0)

                adjt = ipool.tile([128, 36], i32, tag="adjt")
                nc.gpsimd.dma_start(adjt[:, :].rearrange("p (s c) -> p s c", s=4), adj_r[t])

                # counts: cnt[s,i,j] = adj[s,j,i] + (i==j), as f32
                cnt9 = ipool.tile([128, 36], f32, tag="cnt9")
                nc.vector.tensor_copy(
                    cnt9[:, :].rearrange("p (s i j) -> p s i j", s=4, i=3, j=3),
                    adjt[:, :].rearrange("p (s x y) -> p s y x", s=4, x=3, y=3),
                )
                diagv = cnt9[:, :].rearrange("p (s c) -> p s c", s=4)[:, :, 0:9:4]
                nc.vector.tensor_scalar_add(diagv, diagv, 1.0)

                zs1 = [None, None]
                e9 = apool.tile([128, 36], f32, tag="e9")
                for s in range(4):
                    # ---- xT = transpose(x) ; one MM -> z1(+b1) cols 0:192, s6 cols 192:198
                    xt_ps = ps_t.tile([106, 128], f32, tag="ps_tr")
                    nc.tensor.transpose(
                        xt_ps[:, :], xbm[:, 106 * s : 106 * s + 106], c_id[:, :]
                    )
                    xt = xtpool.tile([106, 128], f32, tag="xt")
                    nc.scalar.copy(xt[:, :], xt_ps[:, :])
                    pr, h = s // 2, s % 2
                    if h == 0:
                        zs1[pr] = ps_z.tile([128, 396], f32, tag="ps_zs", name=f"zs1_{pr}")
                    nc.tensor.matmul(
                        zs1[pr][:, 198 * h : 198 * h + 198],
                        xt[0:106, :],
                        c_w1a[0:106, :],
                        start=True,
                        stop=True,
                    )
                    if h == 1:
                        # evacuate the pair's score cols, then e = s_src + s_dst
                        s6sb = apool.tile([128, 12], f32, tag="s6sb")
                        nc.scalar.copy(
                            s6sb[:, :].rearrange("p (s c) -> p s c", s=2),
                            zs1[pr][:, :].rearrange("p (s c) -> p s c", s=2)[:, :, 192:198],
                        )
                        e_v = e9[:, 18 * pr : 18 * pr + 18].rearrange(
                            "p (s i j) -> p s i j", s=2, i=3, j=3
                        )
                        s6v = s6sb[:, :].rearrange("p (s c) -> p s c", s=2)
                        src = s6v[:, :, 0:3].unsqueeze(2).broadcast_to([128, 2, 3, 3])
                        dst = s6v[:, :, 3:6].unsqueeze(3).broadcast_to([128, 2, 3, 3])
                        nc.vector.tensor_tensor(e_v, src, dst, OP.add)

                def attention(zs, e9t, hname):
                    """softmax+aggregate for all 4 subs; returns h tiles [128,192] per sub.
                    zs: two psum tiles [128,396] (cols: per sub 0:192 z(+bias), 192:198 scores)."""
                    el9 = apool.tile([128, 36], f32, tag="el9")
                    # leaky(0.2) on DVE: max(0.2*x, x). (ACT Lrelu alpha is
                    # global per-program; reserved for the MLP's 0.01.)
                    nc.vector.scalar_tensor_tensor(
                        el9[:, :], e9t[:, :], NEG_ATT, e9t[:, :], OP.mult, OP.max
                    )
                    ex9 = apool.tile([128, 36], f32, tag="ex9")
                    nc.scalar.activation(ex9[:, :], el9[:, :], AF.Exp)
                    p9 = apool.tile([128, 36], f32, tag="p9")
                    nc.vector.tensor_tensor(p9[:, :], ex9[:, :], cnt9[:, :], OP.mult)
                    d12 = apool.tile([128, 12], f32, tag="d12")
                    nc.vector.tensor_reduce(
                        d12[:, :],
                        p9[:, :].rearrange("p (si j) -> p si j", j=3),
                        AX.X,
                        OP.add,
                    )
                    r12 = apool.tile([128, 12], f32, tag="r12")
                    nc.vector.reciprocal(r12[:, :], d12[:, :])
                    # pn[s,i,j'] = p[s,i,j'+1] * r[s,i]  (j'=0,1)
                    pn6 = apool.tile([128, 24], f32, tag="pn6")
                    nc.vector.tensor_tensor(
                        pn6[:, :].rearrange("p (si k) -> p si k", k=2),
                        p9[:, :].rearrange("p (si j) -> p si j", j=3)[:, :, 1:3],
                        r12[:, :].unsqueeze(2).broadcast_to([128, 12, 2]),
                        OP.mult,
                    )
                    hs = []
                    for s2 in range(4):
                        pr2, h2 = s2 // 2, s2 % 2
                        zoff = 198 * h2
                        z0 = apool.tile([128, 64], f32, tag="z0")
                        nc.scalar.copy(z0[:, :], zs[pr2][:, zoff : zoff + 64])
                        dz = apool.tile([128, 128], f32, tag="dz")
                        nc.vector.tensor_tensor(
                            dz[:, :].rearrange("p (k d) -> p k d", k=2),
                            zs[pr2][:, zoff + 64 : zoff + 192].rearrange(
                                "p (k d) -> p k d", k=2
                            ),
                            z0[:, :].unsqueeze(1).broadcast_to([128, 2, 64]),
                            OP.subtract,
                        )
                        ht = hpool.tile([128, 192], f32, tag=hname)
                        for i in range(3):
                            c0 = 6 * s2 + 2 * i
                            nc.vector.scalar_tensor_tensor(
                                ht[:, 64 * i : 64 * i + 64],
                                dz[:, 0:64],
                                pn6[:, c0 : c0 + 1],
                                z0[:, :],
                                OP.mult,
                                OP.add,
                            )
                            nc.vector.scalar_tensor_tensor(
                                ht[:, 64 * i : 64 * i + 64],
                                dz[:, 64:128],
                                pn6[:, c0 + 1 : c0 + 2],
                                ht[:, 64 * i : 64 * i + 64],
                                OP.mult,
                                OP.add,
                            )
                        hs.append(ht)
                    return hs

                h1s = attention(zs1, e9, "h1")

                # ---- GAT2: transpose h1, fuse elu(+1) into evacuation, MM2
                zs2 = [None, None]
                e9b = apool.tile([128, 36], f32, tag="e9b")
                for s in range(4):
                    pr, h = s // 2, s % 2
                    h1eT_a = epool.tile([128, 128], f32, tag="h1eT_a")
                    h1eT_b = epool.tile([65, 128], f32, tag="h1eT_b")
                    nc.gpsimd.memset(h1eT_b[64:65, :], 1.0)
                    for piece, (lo, hi) in enumerate([(0, 128), (128, 192)]):
                        w = hi - lo
                        tp = ps_t.tile([128, 128], f32, tag="ps_tr")
                        nc.tensor.transpose(
                            tp[0:w, :], h1s[s][:, lo:hi], c_id[:, :]
                        )
                        te = epool.tile([128, 128], f32, tag="t_exp")
                        nc.scalar.activation(te[0:w, :], tp[0:w, :], AF.Exp)
                        tr = epool.tile([128, 128], f32, tag="t_relu")
                        nc.scalar.activation(tr[0:w, :], tp[0:w, :], AF.Relu)
                        dstv = h1eT_a[:, :] if piece == 0 else h1eT_b[0:64, :]
                        nc.vector.scalar_tensor_tensor(
                            dstv, te[0:w, :], 1.0, tr[0:w, :], OP.min, OP.add
                        )
                    if h == 0:
                        zs2[pr] = ps_z.tile([128, 396], f32, tag="ps_zs", name=f"zs2_{pr}")
                    nc.tensor.matmul(
                        zs2[pr][:, 198 * h : 198 * h + 198],
                        h1eT_a[:, :],
                        c_w2aa[:, :],
                        start=True,
                        stop=False,
                    )
                    nc.tensor.matmul(
                        zs2[pr][:, 198 * h : 198 * h + 198],
                        h1eT_b[:, :],
                        c_w2ab[:, :],
                        start=False,
                        stop=True,
                    )
                    if h == 1:
                        s6sb2 = apool.tile([128, 12], f32, tag="s6sb")
                        nc.scalar.copy(
                            s6sb2[:, :].rearrange("p (s c) -> p s c", s=2),
                            zs2[pr][:, :].rearrange("p (s c) -> p s c", s=2)[:, :, 192:198],
                        )
                        e_v = e9b[:, 18 * pr : 18 * pr + 18].rearrange(
                            "p (s i j) -> p s i j", s=2, i=3, j=3
                        )
                        s6v2 = s6sb2[:, :].rearrange("p (s c) -> p s c", s=2)
                        src = s6v2[:, :, 0:3].unsqueeze(2).broadcast_to([128, 2, 3, 3])
                        dst = s6v2[:, :, 3:6].unsqueeze(3).broadcast_to([128, 2, 3, 3])
                        nc.vector.tensor_tensor(e_v, src, dst, OP.add)

                h2s = attention(zs2, e9b, "h2")

                # ---- MLP ----
                for s in range(4):
                    h2T_a = htpool.tile([128, 128], f32, tag="h2T_a")
                    h2T_b = htpool.tile([64, 128], f32, tag="h2T_b")
                    for piece, (lo, hi) in enumerate([(0, 128), (128, 192)]):
                        w = hi - lo
                        tp = ps_t.tile([128, 128], f32, tag="ps_tr")
                        nc.tensor.transpose(tp[0:w, :], h2s[s][:, lo:hi], c_id[:, :])
                        dstv = h2T_a[:, :] if piece == 0 else h2T_b[:, :]
                        nc.scalar.copy(dstv, tp[0:w, :])
                    u1 = ps_u.tile([128, 256], f32, tag="ps_u")
                    nc.tensor.matmul(u1[:, :], h2T_a[:, :], c_wo1a[:, :], start=True, stop=False)
                    nc.tensor.matmul(u1[:, :], h2T_b[:, :], c_wo1b[:, :], start=False, stop=True)

                    # LN1 + leaky via single ACT op
                    st6 = spool.tile([128, 6], f32, tag="st6")
                    nc.vector.bn_stats(st6[:, :], u1[:, :])
                    st2 = spool.tile([128, 2], f32, tag="st2")
                    nc.vector.bn_aggr(st2[:, :], st6[:, :])
                    lnv = spool.tile([128, 1], f32, tag="lnv")
                    nc.scalar.activation(lnv[:, :], st2[:, 1:2], AF.Ln, bias=c_eps[:, :])
                    rs = spool.tile([128, 1], f32, tag="rs")
                    nc.scalar.activation(rs[:, :], lnv[:, :], AF.Exp, scale=-0.5)
                    nmr = spool.tile([128, 1], f32, tag="nmr")
                    nc.vector.scalar_tensor_tensor(
                        nmr[:, :], st2[:, 0:1], -1.0, rs[:, :], OP.mult, OP.mult
                    )
                    y1 = ypool.tile([128, 256], f32, tag="y1")
                    nc.scalar.activation(
                        y1[:, :], u1[:, :], AF.Lrelu, bias=nmr[:, :], scale=rs[:, :],
                        alpha=NEG_MLP,
                    )

                    u1T_a = htpool.tile([128, 128], f32, tag="u1T_a")
                    u1T_b = htpool.tile([128, 128], f32, tag="u1T_b")
                    for piece in range(2):
                        tp = ps_t.tile([128, 128], f32, tag="ps_tr")
                        nc.tensor.transpose(
                            tp[:, :], y1[:, 128 * piece : 128 * piece + 128], c_id[:, :]
                        )
                        nc.scalar.copy((u1T_a if piece == 0 else u1T_b)[:, :], tp[:, :])
                    u2 = ps_u.tile([128, 128], f32, tag="ps_u")
                    nc.tensor.matmul(u2[:, :], u1T_a[:, :], c_wo2a[:, :], start=True, stop=False)
                    nc.tensor.matmul(u2[:, :], u1T_b[:, :], c_wo2b[:, :], start=False, stop=True)

                    st6b = spool.tile([128, 6], f32, tag="st6")
                    nc.vector.bn_stats(st6b[:, :], u2[:, :])
                    st2b = spool.tile([128, 2], f32, tag="st2")
                    nc.vector.bn_aggr(st2b[:, :], st6b[:, :])
                    lnvb = spool.tile([128, 1], f32, tag="lnv")
                    nc.scalar.activation(lnvb[:, :], st2b[:, 1:2], AF.Ln, bias=c_eps[:, :])
                    rsb = spool.tile([128, 1], f32, tag="rs")
                    nc.scalar.activation(rsb[:, :], lnvb[:, :], AF.Exp, scale=-0.5)
                    nmrb = spool.tile([128, 1], f32, tag="nmr")
                    nc.vector.scalar_tensor_tensor(
                        nmrb[:, :], st2b[:, 0:1], -1.0, rsb[:, :], OP.mult, OP.mult
                    )
                    y2 = ypool.tile([128, 128], f32, tag="y2")
                    nc.scalar.activation(
                        y2[:, :], u2[:, :], AF.Lrelu, bias=nmrb[:, :], scale=rsb[:, :],
                        alpha=NEG_MLP,
                    )

                    # final dot with wo3 (replicated) -> outacc column
                    scr = ypool.tile([128, 128], f32, tag="scr")
                    nc.vector.scalar_tensor_tensor(
                        scr[:, :],
                        y2[:, :],
                        1.0,
                        c_wo3r[:, :],
                        OP.mult,
                        OP.mult,
                        accum_out=outacc[:, 4 * t + s : 4 * t + s + 1],
                    )

            # ---- flush output ----
            out_kp = out.rearrange("(k p) one -> k (p one)", p=128)
            for c0 in range(0, OUTC, 128):
                w = min(128, OUTC - c0)
                tp = ps_t.tile([128, 128], f32, tag="ps_tr")
                nc.tensor.transpose(tp[0:w, :], outacc[:, c0 : c0 + w], c_id[:, :])
                osb = ypool.tile([128, 128], f32, tag="osb")
                nc.scalar.copy(osb[0:w, :], tp[0:w, :])
                nc.gpsimd.dma_start(out_kp[c0 : c0 + w, :], osb[0:w, :])

    if split_waits:
        _split_excess_waits(nc)
    return nc


def _host_tables(W1, a1s, a1d, b1, W2, a2s, a2d, b2, Wo1, bo1, Wo2, bo2, Wo3, bo3):
    """Precompute the device constant tables (all float32 numpy)."""
    f = lambda x: np.asarray(x, dtype=np.float32)
    W1, a1s, a1d, b1 = f(W1), f(a1s), f(a1d), f(b1)
    W2, a2s, a2d, b2 = f(W2), f(a2s), f(a2d), f(b2)
    Wo1, bo1, Wo2, bo2, Wo3, bo3 = f(Wo1), f(bo1), f(Wo2), f(bo2), f(Wo3), f(bo3)

    # w1a: rows = x features in [obs(90) | act(15) | one] order, cols = [z(192)|s6(6)]
    w1a = np.zeros((IN * N + 1, 198), np.float32)
    w1s, w1d = W1 @ a1s, W1 @ a1d
    for n in range(N):
        # x feature (n, j): obs j<30 at row 30n+j ; act j at row 90+5n+j
        rows = np.r_[30 * n : 30 * n + 30, 90 + 5 * n : 90 + 5 * n + 5]
        w1a[rows, 64 * n : 64 * n + 64] = W1
        w1a[rows, 192 + n] = w1s
        w1a[rows, 195 + n] = w1d
    w1a[105, 0:192] = np.tile(b1, 3)  # bias row (scores get no bias)

    # w2a: rows 0..191 block-diag W2 + score cols; row 192 = b2 - elu(+1) correction
    w2a = np.zeros((193, 198), np.float32)
    w2s, w2d = W2 @ a2s, W2 @ a2d
    for n in range(N):
        w2a[64 * n : 64 * n + 64, 64 * n : 64 * n + 64] = W2
        w2a[64 * n : 64 * n + 64, 192 + n] = w2s
        w2a[64 * n : 64 * n + 64, 195 + n] = w2d
    corr = np.zeros(198, np.float32)
    corr[0:192] = np.tile(W2.sum(axis=0), 3)
    corr[192:195] = w2s.sum()
    corr[195:198] = w2d.sum()
    w2a[192, :] = -corr
    w2a[192, 0:192] += np.tile(b2, 3)

    assert np.allclose(bo1, 0) and np.allclose(bo2, 0), "nonzero MLP biases unsupported"
    ident = np.eye(128, dtype=np.float32)
    wo3r = np.tile(Wo3.reshape(1, 128), (128, 1))
    assert np.allclose(bo3, 0), "nonzero bo3 unsupported"
    return dict(
        w1a=w1a,
        w2aa=w2a[0:128],
        w2ab=w2a[128:193],
        wo1a=Wo1[0:128],
        wo1b=Wo1[128:192],
        wo2a=Wo2[0:128],
        wo2b=Wo2[128:256],
        wo3r=wo3r,
        ident=ident,
    )


def kernel(
    obs,
    action,
    adj_matrix,
    W1,
    a1s,
    a1d,
    b1,
    W2,
    a2s,
    a2d,
    b2,
    Wo1,
    bo1,
    g1,
    be1,
    Wo2,
    bo2,
    g2,
    be2,
    Wo3,
    bo3,
    _bc=None,
    _trace=False,
):
    from concourse.bass_utils import run_bass_kernel_spmd

    obs = np.asarray(obs, dtype=np.float32)
    action = np.asarray(action, dtype=np.float32)
    adj = np.ascontiguousarray(np.asarray(adj_matrix, dtype=np.int32)).reshape(-1, 9)
    Btot = obs.shape[0]
    assert np.allclose(np.asarray(g1), 1) and np.allclose(np.asarray(g2), 1), (
        "non-unit LN gains unsupported"
    )
    assert np.allclose(np.asarray(be1), 0) and np.allclose(np.asarray(be2), 0), (
        "nonzero LN betas unsupported"
    )

    Bc = _bc if _bc is not None else Btot // NCORES
    ncores = Btot // Bc
    assert Bc * ncores == Btot and Bc % 512 == 0

    key = (Bc,)
    if key not in _prog_cache:
        _prog_cache[key] = _build_program(Bc)
    nc = _prog_cache[key]

    tables = _host_tables(
        W1, a1s, a1d, b1, W2, a2s, a2d, b2, Wo1, bo1, Wo2, bo2, Wo3, bo3
    )
    in_maps = []
    for c in range(ncores):
        sl = slice(c * Bc, (c + 1) * Bc)
        m = {"obs": obs[sl], "act": action[sl], "adj": adj[sl]}
        m.update(tables)
        in_maps.append(m)

    res = run_bass_kernel_spmd(
        nc, in_maps, core_ids=list(range(ncores)), trace=_trace
    )
    out = np.concatenate([res.results[c]["out"] for c in range(ncores)], axis=0)
    if _trace:
        kernel._last_results = res
    return out.astype(np.float32)
